# revision 1
# baseline (speedup 1.0000x reference)
"""Trainium2 Bass kernel for nn_Attention (GQA attention + pairwise bias).

Sharding: 8 cores, sequence-parallel. Core c owns query rows [256c, 256c+256)
and pairwise rows ip in [64c, 64c+64). k/v are computed replicated (1 KV head).
No collectives; host concatenates the 8 output slices.

Layout strategy (transposed-sim attention):
  - all HBM loads are natural (row-contiguous); f32->bf16 cast during DMA
  - xT via PE transposes; qkv natural; RMSNorm/RoPE on natural tiles
  - q,k transposed (PE) post-norm -> sim computed transposed [j, i]
  - softcap => no max subtraction; softmax denominator via ones-column in v
  - pairwise bias: BN+GELU fused into the transpose-copy (per-partition
    scale/bias on ACT), tiny matmul vs W_bias, expansion matmuls accumulate
    the bias directly into the sim PSUM tiles
  - AV accumulated transposed [dv, i]; normalize by S; output projection
"""
import numpy as np

N = 2048          # tokens
DIM = 1024
H = 8             # query heads
D_QK = 128
D_V = 192
QKV_COLS = H * D_QK + D_QK + D_V   # 1344
PW = 512          # pairwise i/j
C = 128           # pairwise channels
NCORES = 8
NB = N // NCORES            # 256 own tokens per core
NO = NB // 128              # 2 own token chunks
NT = N // 128               # 16 token chunks
DK = DIM // 128             # 8 dim chunks
IPB = PW // NCORES          # 64 own pairwise rows
PWROWS = IPB * PW           # 32768 flattened own pairwise rows
RT = PWROWS // 128          # 256 pairwise row tiles
SCALE = 64 ** -0.5
SOFTCLAMP = 5.0
RMS_EPS = 1.1920929e-07
BN_EPS = 1e-05
PI_2 = 1.5707963267948966


def build_kernel():
    from concourse import bass, bacc, mybir
    from concourse.tile import TileContext
    from concourse.masks import make_identity

    f32 = mybir.dt.float32
    b16 = mybir.dt.bfloat16
    AF = mybir.ActivationFunctionType
    OP = mybir.AluOpType

    nc = bacc.Bacc()
    dp = lambda name, shape: nc.declare_dram_parameter(name, shape, f32, isOutput=False)
    x_d = dp("x", [N, DIM])
    xo_d = dp("x_own", [NB, DIM])
    rot_d = dp("rotary", [N, D_QK])
    roto_d = dp("rotary_own", [NB, D_QK])
    pw_d = dp("pairwise", [PWROWS, C])
    wqkv_d = dp("W_qkv", [DIM, QKV_COLS])
    qw_d = dp("q_norm_w", [D_QK])
    kw_d = dp("k_norm_w", [D_QK])
    vw_d = dp("v_norm_w", [D_V])
    bng_d = dp("bn_gamma", [C])
    bnb_d = dp("bn_beta", [C])
    bnv_d = dp("bn_running_var", [C])
    wb_d = dp("W_bias", [C, H])
    wout_d = dp("W_out", [H * D_V, DIM])
    out_d = nc.declare_dram_parameter("out", [NB, DIM], f32, isOutput=True)

    with TileContext(nc) as tc:
        import contextlib
        with contextlib.ExitStack() as ctx:
            const = ctx.enter_context(tc.tile_pool(name="const", bufs=1))
            persist = ctx.enter_context(tc.tile_pool(name="persist", bufs=1))
            pwpool = ctx.enter_context(tc.tile_pool(name="pwpool", bufs=6))

            # ---- constants ----
            id128 = const.tile([128, 128], b16)
            make_identity(nc, id128)
            # E2 expansion matrices: E2[a][p, 32j+r] = 1 iff p == 32a + j
            E2 = []
            for a in range(4):
                e = const.tile([128, 128], b16, tag=f"E2_{a}", name=f"E2_{a}")
                ev = e.rearrange("p (j r) -> p j r", r=4)
                nc.gpsimd.memset(e, 0.0)
                nc.gpsimd.affine_select(
                    out=ev, in_=ev, compare_op=OP.not_equal, fill=1.0,
                    base=-32 * a, pattern=[[-1, 32], [0, 4]], channel_multiplier=1)
                E2.append(e)
            cPI2 = const.tile([128, 1], f32)
            nc.vector.memset(cPI2, PI_2)
            cEPS = const.tile([128, 1], f32)
            nc.vector.memset(cEPS, RMS_EPS)
            wbias = const.tile([C, H], b16)
            nc.gpsimd.dma_start(out=wbias, in_=wb_d[:, :])
            # per-partition bn scale/beta vectors [128,1]
            bng = const.tile([C, 1], f32)
            bnb = const.tile([C, 1], f32)
            bnv = const.tile([C, 1], f32)
            for t, d in ((bng, bng_d), (bnb, bnb_d), (bnv, bnv_d)):
                nc.sync.dma_start(out=t, in_=d.rearrange("(c one) -> c one", one=1))
            # bnscale = (gamma+1)*sqrt(C)/sqrt(max(var, BN_EPS))
            bnsc = const.tile([C, 1], f32)
            nc.vector.tensor_scalar_max(out=bnsc, in0=bnv, scalar1=BN_EPS)
            nc.scalar.sqrt(bnsc, bnsc)
            nc.vector.reciprocal(bnsc, bnsc)
            tmpg = const.tile([C, 1], f32)
            nc.vector.tensor_scalar_add(out=tmpg, in0=bng, scalar1=1.0)
            nc.vector.tensor_mul(bnsc, bnsc, tmpg)
            nc.scalar.mul(bnsc, bnsc, float(np.sqrt(C)))
            # norm-weight broadcast tiles
            kw_bc = const.tile([128, D_QK], f32)
            qw_bc = const.tile([128, D_QK], f32)
            vw_bc = const.tile([128, D_V], f32)
            for t, d, w in ((kw_bc, kw_d, D_QK), (qw_bc, qw_d, D_QK), (vw_bc, vw_d, D_V)):
                dap = d.ap()
                src = bass.AP(tensor=dap.tensor, offset=dap.offset, ap=[[0, 128], [1, w]])
                nc.gpsimd.dma_start(out=t, in_=src)
            # swapped-half weight tiles (for rotate_half * sin)
            kw_sw = const.tile([128, D_QK], f32)
            qw_sw = const.tile([128, D_QK], f32)
            for sw, bc in ((kw_sw, kw_bc), (qw_sw, qw_bc)):
                nc.vector.tensor_copy(out=sw[:, 0:64], in_=bc[:, 64:128])
                nc.vector.tensor_copy(out=sw[:, 64:128], in_=bc[:, 0:64])

            # ---- persistent activation buffers ----
            kT = persist.tile([128, N], b16)            # [d, j]
            qT_all = persist.tile([128, H, NB], b16)
            qT = [qT_all[:, h, :] for h in range(H)]
            v_aug = [persist.tile([128, D_V + 1], b16, tag=f"va{t}", name=f"va{t}") for t in range(NT)]

            # pairwise prefetch (bf16 cast during DMA), 6 of 8 chunks in flight
            pw_view = pw_d.rearrange("(a p) c -> p a c", p=128)  # [128, 256, 128]
            pw_sb = []
            for ch in range(8):
                t = pwpool.tile([128, RT // 8, C], b16, tag="pw", name=f"pw{ch}")
                nc.gpsimd.dma_start(out=t, in_=pw_view[:, 32 * ch:32 * (ch + 1), :])
                pw_sb.append(t)

            # ================= stages A-C =================
            with contextlib.ExitStack() as cctx:
                cpool = cctx.enter_context(tc.tile_pool(name="cpool", bufs=1))
                xpool = cctx.enter_context(tc.tile_pool(name="xpool", bufs=2))
                spool = cctx.enter_context(tc.tile_pool(name="spool", bufs=3))
                vpool = cctx.enter_context(tc.tile_pool(name="vpool", bufs=4))
                pst_p = cctx.enter_context(tc.tile_pool(name="pst", bufs=2, space="PSUM"))
                pkv_p = cctx.enter_context(tc.tile_pool(name="pkv", bufs=2, space="PSUM"))
                pq_p = cctx.enter_context(tc.tile_pool(name="pq", bufs=1, space="PSUM"))

                # load W_qkv bf16
                wqkv = [cpool.tile([128, QKV_COLS], b16, tag=f"wqkv{k}", name=f"wqkv{k}") for k in range(DK)]
                wq_v = wqkv_d.rearrange("(a p) c -> p a c", p=128)
                for k in range(DK):
                    nc.gpsimd.dma_start(out=wqkv[k], in_=wq_v[:, k, :])

                # load x (bf16) and transpose -> xT
                xT_all = cpool.tile([128, DK, N], b16)
                xTo_all = cpool.tile([128, DK, NB], b16)
                xT = [xT_all[:, k, :] for k in range(DK)]
                xTo = [xTo_all[:, k, :] for k in range(DK)]
                x_v = x_d.rearrange("(a p) c -> p a c", p=128)   # [128, 16, 1024]
                for g in range(4):
                    xn = xpool.tile([128, 4, DIM], b16, tag="xn", name="xn")
                    nc.gpsimd.dma_start(out=xn, in_=x_v[:, 4 * g:4 * (g + 1), :])
                    for a in range(4):
                        tcn = 4 * g + a
                        ps = pst_p.tile([128, 1024], b16, tag="pst", name="pst")
                        for k in range(DK):
                            nc.tensor.transpose(ps[:, 128 * k:128 * (k + 1)],
                                                xn[:, a, 128 * k:128 * (k + 1)], id128)
                        dst = bass.AP(tensor=xT_all.tensor, offset=xT_all.offset + 128 * tcn,
                                      ap=[xT_all.ap[0], [N, DK], [1, 128]])
                        nc.vector.tensor_copy(out=dst, in_=ps)
                xo_v = xo_d.rearrange("(a p) c -> p a c", p=128)  # [128, 2, 1024]
                xon = xpool.tile([128, NO, DIM], b16, tag="xon", name="xon")
                nc.gpsimd.dma_start(out=xon, in_=xo_v[:, :, :])
                for a in range(NO):
                    ps = pst_p.tile([128, 1024], b16, tag="pst", name="pst")
                    for k in range(DK):
                        nc.tensor.transpose(ps[:, 128 * k:128 * (k + 1)],
                                            xon[:, a, 128 * k:128 * (k + 1)], id128)
                    dst = bass.AP(tensor=xTo_all.tensor, offset=xTo_all.offset + 128 * a,
                                  ap=[xTo_all.ap[0], [NB, DK], [1, 128]])
                    nc.vector.tensor_copy(out=dst, in_=ps)

                # rotary -> weighted cos/sin tiles (k: all chunks, q: own chunks)
                wcos_k, wsin_k = [], []
                rot_v = rot_d.rearrange("(a p) c -> p a c", p=128)
                rotn = cpool.tile([128, NT, D_QK], f32)
                nc.sync.dma_start(out=rotn, in_=rot_v[:, :, :])
                roto = cpool.tile([128, NO, D_QK], f32)
                nc.sync.dma_start(out=roto, in_=roto_d.rearrange("(a p) c -> p a c", p=128)[:, :, :])
                for t in range(NT):
                    cs = cpool.tile([128, D_QK], f32, tag=f"wck{t}", name=f"wck{t}")
                    sn = cpool.tile([128, D_QK], f32, tag=f"wsk{t}", name=f"wsk{t}")
                    nc.scalar.activation(out=cs, in_=rotn[:, t, :], func=AF.Sin, bias=cPI2)
                    nc.scalar.activation(out=sn, in_=rotn[:, t, :], func=AF.Sin)
                    nc.vector.tensor_mul(cs, cs, kw_bc)
                    nc.vector.tensor_mul(sn, sn, kw_sw)
                    wcos_k.append(cs)
                    wsin_k.append(sn)
                wcos_q, wsin_q = [], []
                for t in range(NO):
                    cs = cpool.tile([128, D_QK], f32, tag=f"wcq{t}", name=f"wcq{t}")
                    sn = cpool.tile([128, D_QK], f32, tag=f"wsq{t}", name=f"wsq{t}")
                    nc.scalar.activation(out=cs, in_=roto[:, t, :], func=AF.Sin, bias=cPI2)
                    nc.scalar.activation(out=sn, in_=roto[:, t, :], func=AF.Sin)
                    nc.vector.tensor_mul(cs, cs, qw_bc)
                    nc.vector.tensor_mul(sn, sn, qw_sw)
                    wcos_q.append(cs)
                    wsin_q.append(sn)

                def norm_rope(src_ap, d, wcos, wsin, ps_dst, extra_scale=None):
                    """RMSNorm(+w fused into wcos/wsin) + RoPE + PE-transpose
                    into the given psum slice (caller flushes batched)."""
                    sq = spool.tile([128, d], b16, tag="sq", name="sq")
                    ss = vpool.tile([128, 1], f32, tag="ss", name="ss")
                    nc.scalar.activation(out=sq, in_=src_ap, func=AF.Square, accum_out=ss)
                    sd = vpool.tile([128, 1], f32, tag="sd", name="sd")
                    nc.scalar.activation(out=sd, in_=ss, func=AF.Sqrt, scale=1.0 / d, bias=cEPS)
                    rs = vpool.tile([128, 1], f32, tag="rs", name="rs")
                    nc.vector.reciprocal(rs, sd)
                    if extra_scale is not None:
                        nc.scalar.mul(rs, rs, extra_scale)
                    hd = d // 2
                    m1 = spool.tile([128, d], f32, tag="m1", name="m1")
                    nc.vector.scalar_tensor_tensor(out=m1, in0=src_ap, scalar=rs,
                                                   in1=wcos, op0=OP.mult, op1=OP.mult)
                    t2 = spool.tile([128, hd], f32, tag="t2", name="t2")
                    rb = spool.tile([128, d], b16, tag="rb", name="rb")
                    nc.vector.scalar_tensor_tensor(out=t2, in0=src_ap[:, hd:d], scalar=rs,
                                                   in1=wsin[:, 0:hd], op0=OP.mult, op1=OP.mult)
                    nc.vector.tensor_sub(rb[:, 0:hd], m1[:, 0:hd], t2)
                    t3 = spool.tile([128, hd], f32, tag="t3", name="t3")
                    nc.vector.scalar_tensor_tensor(out=t3, in0=src_ap[:, 0:hd], scalar=rs,
                                                   in1=wsin[:, hd:d], op0=OP.mult, op1=OP.mult)
                    nc.vector.tensor_add(rb[:, hd:d], m1[:, hd:d], t3)
                    nc.tensor.transpose(ps_dst, rb, id128)

                # k/v for all chunks
                for t in range(NT):
                    ps_kv = pkv_p.tile([128, 320], f32, tag="pskv", name="pskv")
                    # matmul(out, lhsT, rhs) => out = lhsT.T @ rhs:
                    # kv[tok, col] = (xT chunk).T @ W chunk
                    for k in range(DK):
                        nc.tensor.matmul(ps_kv, xT[k][:, 128 * t:128 * (t + 1)],
                                         wqkv[k][:, 1024:1344],
                                         start=(k == 0), stop=(k == DK - 1))
                    if t % 4 == 0:
                        ps_k4 = pst_p.tile([128, 512], b16, tag="pstk", name="pstk")
                    norm_rope(ps_kv[:, 0:128], D_QK, wcos_k[t], wsin_k[t],
                              ps_k4[:, 128 * (t % 4):128 * (t % 4 + 1)])
                    if t % 4 == 3:
                        nc.vector.tensor_copy(out=kT[:, 512 * (t // 4):512 * (t // 4 + 1)],
                                              in_=ps_k4)
                    ssv = vpool.tile([128, 1], f32, tag="ssv", name="ssv")
                    sqv = spool.tile([128, D_V], b16, tag="sqv", name="sqv")
                    nc.scalar.activation(out=sqv, in_=ps_kv[:, 128:320], func=AF.Square, accum_out=ssv)
                    sdv = vpool.tile([128, 1], f32, tag="sdv", name="sdv")
                    nc.scalar.activation(out=sdv, in_=ssv, func=AF.Sqrt, scale=1.0 / D_V, bias=cEPS)
                    rv = vpool.tile([128, 1], f32, tag="rv", name="rv")
                    nc.vector.reciprocal(rv, sdv)
                    nc.vector.scalar_tensor_tensor(out=v_aug[t][:, 0:D_V], in0=ps_kv[:, 128:320],
                                                   scalar=rv, in1=vw_bc, op0=OP.mult, op1=OP.mult)
                    nc.vector.memset(v_aug[t][:, D_V:D_V + 1], 1.0)

                # q for own chunks
                for oc in range(NO):
                    psq0 = pq_p.tile([128, 512], f32, tag="psq0", name="psq0")
                    psq1 = pq_p.tile([128, 512], f32, tag="psq1", name="psq1")
                    for k in range(DK):
                        nc.tensor.matmul(psq0, xTo[k][:, 128 * oc:128 * (oc + 1)],
                                         wqkv[k][:, 0:512], start=(k == 0), stop=(k == DK - 1))
                        nc.tensor.matmul(psq1, xTo[k][:, 128 * oc:128 * (oc + 1)],
                                         wqkv[k][:, 512:1024], start=(k == 0), stop=(k == DK - 1))
                    ps_q8 = pst_p.tile([128, 1024], b16, tag="pst", name="ps_q8")
                    for h in range(H):
                        src = psq0 if h < 4 else psq1
                        col = (h % 4) * 128
                        norm_rope(src[:, col:col + 128], D_QK,
                                  wcos_q[oc], wsin_q[oc],
                                  ps_q8[:, 128 * h:128 * (h + 1)],
                                  extra_scale=SCALE)
                    dstq = bass.AP(tensor=qT_all.tensor, offset=qT_all.offset + 128 * oc,
                                   ap=[qT_all.ap[0], [NB, H], [1, 128]])
                    nc.vector.tensor_copy(out=dstq, in_=ps_q8)

            # ================= stage D: pairwise bias =================
            # BMTexp[jpb] layout [jp_local(128), ip(64), r(4), h(8)] f32
            with contextlib.ExitStack() as dctx:
                dpool = dctx.enter_context(tc.tile_pool(name="dpool", bufs=1))
                BMTexp = [dpool.tile([128, IPB, 4, H], b16, tag=f"bmt{j}", name=f"bmt{j}") for j in range(4)]
                with tc.tile_pool(name="pd", bufs=2, space="PSUM") as pd_p, \
                     tc.tile_pool(name="pbias", bufs=1, space="PSUM") as pb_p, \
                     tc.tile_pool(name="pgp", bufs=3) as pg_p:
                    bias_ps = [pb_p.tile([128, IPB * H], f32, tag=f"bps{j}", name=f"bps{j}") for j in range(4)]
                    for g in range(RT // 8):
                        ps = pd_p.tile([128, 1024], b16, tag="psd", name="psd")
                        for j in range(8):
                            rt = 8 * g + j
                            ch, loc = rt // 32, rt % 32
                            nc.tensor.transpose(ps[:, 128 * j:128 * (j + 1)],
                                                pw_sb[ch][:, loc, :], id128)
                        pg = pg_p.tile([C, 1024], b16, tag="pg", name="pg")
                        nc.scalar.activation(out=pg, in_=ps, func=AF.Gelu, scale=bnsc, bias=bnb)
                        for j in range(8):
                            rt = 8 * g + j
                            ip, jpb = rt // 4, rt % 4
                            nc.tensor.matmul(bias_ps[jpb][:, H * ip:H * (ip + 1)],
                                             pg[:, 128 * j:128 * (j + 1)], wbias,
                                             start=True, stop=True)
                    for jpb in range(4):
                        for r in range(4):
                            dst = bass.AP(tensor=BMTexp[jpb].tensor, offset=BMTexp[jpb].offset + 8 * r,
                                          ap=[BMTexp[jpb].ap[0], [32, IPB], [1, H]])
                            nc.vector.tensor_copy(out=dst, in_=bias_ps[jpb])

                # ================= stage E: attention =================
                with contextlib.ExitStack() as ectx:
                    epool = ectx.enter_context(tc.tile_pool(name="epool", bufs=1))
                    expp = ectx.enter_context(tc.tile_pool(name="expp", bufs=2 * NT))
                    tanp = ectx.enter_context(tc.tile_pool(name="tanp", bufs=3))
                    nrmp = ectx.enter_context(tc.tile_pool(name="nrmp", bufs=2))

                    woutA = [epool.tile([128, DIM], b16, tag=f"wA{h}", name=f"wA{h}") for h in range(H)]
                    woutB = [epool.tile([64, DIM], b16, tag=f"wB{h}", name=f"wB{h}") for h in range(H)]
                    for h in range(H):
                        nc.gpsimd.dma_start(out=woutA[h], in_=wout_d[192 * h:192 * h + 128, :])
                        nc.gpsimd.dma_start(out=woutB[h], in_=wout_d[192 * h + 128:192 * (h + 1), :])
                    oT0 = [epool.tile([128, NB], b16, tag=f"oT0{h}", name=f"oT0{h}") for h in range(H)]
                    oT1 = [epool.tile([64, NB], b16, tag=f"oT1{h}", name=f"oT1{h}") for h in range(H)]

                    estk = ectx.enter_context(contextlib.ExitStack())
                    psim = estk.enter_context(tc.tile_pool(name="psim", bufs=2, space="PSUM"))
                    pav = estk.enter_context(tc.tile_pool(name="pav", bufs=1, space="PSUM"))
                    for pair in range(H // 2):
                        hs = (2 * pair, 2 * pair + 1)
                        po = {}
                        for h in hs:
                            po[(h, 0)] = pav.tile([128, NB], f32, tag=f"po{h % 2}0", name=f"po{h % 2}0")
                            po[(h, 1)] = pav.tile([128, NB], f32, tag=f"po{h % 2}1", name=f"po{h % 2}1")
                        import os as _os
                        SIMFUSE = _os.environ.get("KSIMFUSE", "1") == "1"
                        expT = {h: [] for h in hs}
                        for jc in range(NT):
                            a, jpb = jc % 4, jc // 4
                            if SIMFUSE:
                                sims = psim.tile([128, 2 * NB], f32, tag="sim", name="sim")
                                svs = [sims[:, 0:NB], sims[:, NB:2 * NB]]
                            else:
                                svs = [psim.tile([128, NB], f32, tag=f"sim{u}", name=f"sim{u}")
                                       for u in range(2)]
                            for u, h in enumerate(hs):
                                nc.tensor.matmul(svs[u], kT[:, 128 * jc:128 * (jc + 1)], qT[h],
                                                 start=True, stop=False)
                                rhs = bass.AP(tensor=BMTexp[jpb].tensor,
                                              offset=BMTexp[jpb].offset + h,
                                              ap=[BMTexp[jpb].ap[0], [8, NB]])
                                nc.tensor.matmul(svs[u], E2[a], rhs,
                                                 start=False, stop=True,
                                                 skip_group_check=True)
                            if SIMFUSE:
                                tn = tanp.tile([128, 2 * NB], b16, tag="tanh", name="tanh")
                                nc.scalar.activation(out=tn, in_=sims, func=AF.Tanh,
                                                     scale=1.0 / SOFTCLAMP)
                                ex = expp.tile([128, 2 * NB], b16, tag="expT", name="expT")
                                nc.scalar.activation(out=ex, in_=tn, func=AF.Exp, scale=SOFTCLAMP)
                                expT[hs[0]].append(ex[:, 0:NB])
                                expT[hs[1]].append(ex[:, NB:2 * NB])
                            else:
                                ex2 = expp.tile([128, 2 * NB], b16, tag="expT", name="expT")
                                for u, h in enumerate(hs):
                                    tn = tanp.tile([128, NB], b16, tag="tanh", name="tanh")
                                    nc.scalar.activation(out=tn, in_=svs[u], func=AF.Tanh,
                                                         scale=1.0 / SOFTCLAMP)
                                    nc.scalar.activation(out=ex2[:, NB * u:NB * (u + 1)], in_=tn,
                                                         func=AF.Exp, scale=SOFTCLAMP)
                                expT[hs[0]].append(ex2[:, 0:NB])
                                expT[hs[1]].append(ex2[:, NB:2 * NB])
                        for jc in range(NT):
                            for dvh in range(2):
                                lhs = v_aug[jc][:, 0:128] if dvh == 0 else v_aug[jc][:, 128:193]
                                for h in hs:
                                    nc.tensor.matmul(po[(h, dvh)][0:(128 if dvh == 0 else 65), :],
                                                     lhs, expT[h][jc],
                                                     start=(jc == 0), stop=(jc == NT - 1))
                        for h in hs:
                            s_sb = nrmp.tile([1, NB], f32, tag="ssb", name="ssb")
                            nc.vector.tensor_copy(out=s_sb, in_=po[(h, 1)][64:65, :])
                            rs = nrmp.tile([1, NB], f32, tag="rsb", name="rsb")
                            nc.vector.reciprocal(rs, s_sb)
                            rsb = nrmp.tile([128, NB], f32, tag="rsbc", name="rsbc")
                            nc.gpsimd.partition_broadcast(rsb, rs)
                            nc.vector.tensor_mul(oT0[h], po[(h, 0)], rsb)
                            nc.vector.tensor_mul(oT1[h], po[(h, 1)][0:64, :], rsb[0:64, :])

                    # ================= stage F: output projection =================
                    estk.close()  # free attention PSUM pools before stage F
                    with tc.tile_pool(name="pf", bufs=2, space="PSUM") as pf_p, \
                         tc.tile_pool(name="fo", bufs=2) as fo_p:
                        for ic in range(NO):
                            for nh in range(2):
                                pf = pf_p.tile([128, 512], f32, tag="pf", name="pf")
                                for h in range(H):
                                    nc.tensor.matmul(pf, oT0[h][:, 128 * ic:128 * (ic + 1)],
                                                     woutA[h][:, 512 * nh:512 * (nh + 1)],
                                                     start=(h == 0), stop=False)
                                    nc.tensor.matmul(pf, oT1[h][:, 128 * ic:128 * (ic + 1)],
                                                     woutB[h][:, 512 * nh:512 * (nh + 1)],
                                                     start=False, stop=(h == H - 1))
                                osb = fo_p.tile([128, 512], f32, tag="osb", name="osb")
                                nc.scalar.copy(out=osb, in_=pf)
                                nc.sync.dma_start(
                                    out=out_d[128 * ic:128 * (ic + 1), 512 * nh:512 * (nh + 1)],
                                    in_=osb)
    nc.compile()
    return nc


_NC = None


def kernel(x, pairwise, rotary_emb, W_qkv, q_norm_w, k_norm_w, v_norm_w,
           bn_gamma, bn_beta, bn_running_var, W_bias, W_out):
    global _NC
    from concourse.bass_utils import run_bass_kernel_spmd
    if _NC is None:
        _NC = build_kernel()
    f = lambda a: np.ascontiguousarray(np.asarray(a), dtype=np.float32)
    xf = f(x)[0]
    pwf = f(pairwise)[0].reshape(PW * PW, C)
    rotf = f(rotary_emb)
    base = {
        "x": xf, "rotary": rotf, "W_qkv": f(W_qkv),
        "q_norm_w": f(q_norm_w), "k_norm_w": f(k_norm_w), "v_norm_w": f(v_norm_w),
        "bn_gamma": f(bn_gamma), "bn_beta": f(bn_beta),
        "bn_running_var": f(bn_running_var), "W_bias": f(W_bias), "W_out": f(W_out),
    }
    in_maps = []
    for c in range(NCORES):
        m = dict(base)
        m["x_own"] = xf[NB * c:NB * (c + 1)]
        m["rotary_own"] = rotf[NB * c:NB * (c + 1)]
        m["pairwise"] = pwf[PWROWS * c:PWROWS * (c + 1)]
        in_maps.append(m)
    res = run_bass_kernel_spmd(_NC, in_maps, list(range(NCORES)))
    out = np.concatenate([res.results[c]["out"] for c in range(NCORES)], axis=0)
    return out[None].astype(np.float32)

